# revision 1
# baseline (speedup 1.0000x reference)
"""Causal self-attention with RoPE on 8 TRN2 NeuronCores.

Sharding: tensor-parallel over heads (H=8 -> 1 head per core).
Each core computes, for its head h:
    q,k,v projections (bf16 matmuls, fp32 PSUM)  ->  RoPE (DVE, fp32 tables)
    S^T blocks (j,i) via K=64 row-paired concurrent matmuls
    P^T = exp(S^T/8) on ACT (bf16 out), causal diag masking via affine_select
    y_u^T = [v | ones]^T-weighted PV matmuls  (row 64 = softmax denominator)
    out_u = y_u @ Wp_h^T on-device; host computes sum_h out_u_h / colsum_h.
"""
import sys

sys.path.insert(0, "/opt/trn_rl_repo")

import numpy as np
import ml_dtypes

import concourse.bass as bass
import concourse.mybir as mybir
import concourse.tile as tile
from concourse.bass_utils import run_bass_kernel_spmd

B, T, C, H = 1, 4096, 512, 8
HS = C // H  # 64
NCORES = 8
TB = 512           # t-block width for projections / i-block width for attention
NTB = T // TB      # 8
JC = 128           # j-chunk width
NJC = T // JC      # 32

_ctr = [0]


def _legalize_waits(nc):
    """This walrus build accepts at most one sem-wait command per hw
    instruction; move extra waits onto same-engine NoOps inserted before."""
    for f in nc.m.functions:
        for bb in f.blocks:
            insts = bb.instructions
            out = []
            for inst in insts:
                si = inst.sync_info
                if si is not None and len(si.on_wait) > 1:
                    waits = list(si.on_wait)
                    for w in waits[:-1]:
                        _ctr[0] += 1
                        nop = mybir.InstNoOp(name=f"I-waitsplit-{_ctr[0]}")
                        nop.engine = inst.engine
                        nop.sync_info = mybir.SyncInfo(on_wait=[w], on_update=[])
                        out.append(nop)
                    inst.sync_info = mybir.SyncInfo(
                        on_wait=[waits[-1]], on_update=list(si.on_update)
                    )
                out.append(inst)
            insts[:] = out
    return nc


def _build_nc(trace_scopes=False):
    nc = bass.Bass()
    f32 = mybir.dt.float32
    bf16 = mybir.dt.bfloat16

    xt_in = nc.declare_dram_parameter("xt", [C, T], bf16, isOutput=False)
    wqk_in = nc.declare_dram_parameter("wqk", [C, 128], bf16, isOutput=False)
    wqks_in = nc.declare_dram_parameter("wqks", [C, 128], bf16, isOutput=False)
    wv_in = nc.declare_dram_parameter("wv", [C, HS], bf16, isOutput=False)
    wp_in = nc.declare_dram_parameter("wp", [HS, C], bf16, isOutput=False)
    cc_in = nc.declare_dram_parameter("cc", [128, T], f32, isOutput=False)
    ss_in = nc.declare_dram_parameter("ss", [128, T], f32, isOutput=False)
    out_u = nc.declare_dram_parameter("out_u", [T, C], bf16, isOutput=True)
    cs_out = nc.declare_dram_parameter("cs", [1, T], f32, isOutput=True)

    Exp = mybir.ActivationFunctionType.Exp

    with tile.TileContext(nc) as tc:
        with (
            tc.tile_pool(name="big", bufs=1) as big,
            tc.tile_pool(name="ropet", bufs=3) as ropet,
            tc.tile_pool(name="ptp", bufs=10) as ptp,
            tc.tile_pool(name="ytsb", bufs=2) as ytsb,
            tc.tile_pool(name="outp", bufs=3) as outp,
        ):
            # ---- resident inputs ----
            xt_sb = big.tile([128, 4, T], bf16)
            _xt_r = xt_in.ap().rearrange("(n p) t -> p n t", p=128)
            for _c in range(8):
                _t0 = _c * (T // 8)
                nc.sync.dma_start(out=xt_sb[:, :, _t0:_t0 + T // 8],
                                  in_=_xt_r[:, :, _t0:_t0 + T // 8])
            wqk_sb = big.tile([128, 4, 128], bf16)
            nc.sync.dma_start(out=wqk_sb, in_=wqk_in.ap().rearrange("(n p) m -> p n m", p=128))
            wqks_sb = big.tile([128, 4, 128], bf16)
            nc.sync.dma_start(out=wqks_sb, in_=wqks_in.ap().rearrange("(n p) m -> p n m", p=128))
            wv_sb = big.tile([128, 4, HS], bf16)
            nc.sync.dma_start(out=wv_sb, in_=wv_in.ap().rearrange("(n p) m -> p n m", p=128))
            wp_sb = big.tile([HS, C], bf16)
            nc.sync.dma_start(out=wp_sb, in_=wp_in.ap())
            cc_sb = big.tile([128, T], f32)
            ss_sb = big.tile([128, T], f32)
            for _c in range(4):
                _t0 = _c * (T // 4)
                nc.sync.dma_start(out=cc_sb[:, _t0:_t0 + T // 4], in_=cc_in.ap()[:, _t0:_t0 + T // 4])
                nc.sync.dma_start(out=ss_sb[:, _t0:_t0 + T // 4], in_=ss_in.ap()[:, _t0:_t0 + T // 4])

            qkr = big.tile([128, T], bf16)    # rows 0:64 = q_rot^T, 64:128 = k_rot^T
            krqr = big.tile([128, T], bf16)   # rows 0:64 = k_rot^T, 64:128 = q_rot^T
            v_ones = big.tile([128, NJC, HS + 1], bf16)
            nc.vector.memset(v_ones[:, :, HS], 1.0)
            cs_sb = big.tile([1, T], f32)

            # ---- phase A: qkv projections + rope ----
            with tc.tile_pool(name="qkp", bufs=3, space="PSUM") as qkp:
              for tb in range(NTB):
                  tc0 = tb * TB
                  qk_ps = qkp.tile([128, TB], f32, tag="qk")
                  for cn in range(4):
                      nc.tensor.matmul(qk_ps, wqk_sb[:, cn, :], xt_sb[:, cn, tc0:tc0 + TB],
                                       start=(cn == 0), stop=(cn == 3))
                  qks_ps = qkp.tile([128, TB], f32, tag="qk")
                  for cn in range(4):
                      nc.tensor.matmul(qks_ps, wqks_sb[:, cn, :], xt_sb[:, cn, tc0:tc0 + TB],
                                       start=(cn == 0), stop=(cn == 3))
                  t1 = ropet.tile([128, TB], f32, tag="rt")
                  nc.vector.tensor_mul(t1, qks_ps, ss_sb[:, tc0:tc0 + TB])
                  t2 = ropet.tile([128, TB], f32, tag="rt")
                  nc.vector.tensor_mul(t2, qk_ps, cc_sb[:, tc0:tc0 + TB])
                  nc.vector.tensor_add(qkr[:, tc0:tc0 + TB], t2, t1)
                  # swapped duplicate for the row-paired S^T matmuls
                  nc.sync.dma_start(out=krqr[0:64, tc0:tc0 + TB], in_=qkr[64:128, tc0:tc0 + TB])
                  nc.sync.dma_start(out=krqr[64:128, tc0:tc0 + TB], in_=qkr[0:64, tc0:tc0 + TB])
                  # v in (t, d) layout
                  for t4 in range(4):
                      j = tb * 4 + t4
                      p0 = tc0 + t4 * 128
                      v_ps = qkp.tile([128, HS], f32, tag="vp", bufs=2)
                      for cn in range(4):
                          nc.tensor.matmul(v_ps, xt_sb[:, cn, p0:p0 + 128], wv_sb[:, cn, :],
                                           start=(cn == 0), stop=(cn == 3))
                      nc.vector.tensor_copy(v_ones[:, j, 0:HS], v_ps)

            # ---- phase B: attention + c_proj ----
            with (
                tc.tile_pool(name="stp", bufs=6, space="PSUM") as stp,
                tc.tile_pool(name="ytp", bufs=1, space="PSUM") as ytp,
                tc.tile_pool(name="opp", bufs=1, space="PSUM") as opp,
            ):
              for ib in range(NTB):
                  i0 = ib * TB
                  nj = 4 * ib + 4
                  yt_ps = ytp.tile([128, TB], f32, tag="yt")
                  pend = []  # (pt, j) waiting for their PV matmul

                  def flush_pv(n):
                      while len(pend) > n:
                          pt_, j_ = pend.pop(0)
                          v0_ = max(0, j_ * JC - i0)
                          nc.tensor.matmul(yt_ps[0:HS + 1, v0_:TB], v_ones[:, j_, :],
                                           pt_[:, v0_:TB],
                                           start=(j_ == 0), stop=(j_ == nj - 1),
                                           skip_group_check=True)

                  for m in range(nj // 2):
                      j_e, j_o = 2 * m, 2 * m + 1
                      ve = max(0, j_e * JC - i0)
                      vo = max(0, j_o * JC - i0)
                      st_e = stp.tile([128, TB], f32, tag="st")
                      nc.tensor.matmul(st_e[:, ve:TB], krqr[0:64, j_e * JC:(j_e + 1) * JC],
                                       qkr[0:64, i0 + ve:i0 + TB], tile_position=(0, 0))
                      st_o = stp.tile([128, TB], f32, tag="st")
                      nc.tensor.matmul(st_o[:, vo:TB], qkr[64:128, j_o * JC:(j_o + 1) * JC],
                                       krqr[64:128, i0 + vo:i0 + TB], tile_position=(64, 0))
                      for st, j in ((st_e, j_e), (st_o, j_o)):
                          pt = ptp.tile([128, TB], mybir.dt.bfloat16, tag="pt")
                          v0 = max(0, j * JC - i0)  # cols < v0 are fully masked
                          if v0 > 0:
                              nc.vector.memset(pt[:, 0:v0], 0.0)
                          nc.scalar.activation(pt[:, v0:TB], st[:, v0:TB], Exp, scale=0.125)
                          if j * JC + JC - 1 > i0:  # diagonal band needs elementwise mask
                              b0, b1 = v0, min(TB, v0 + JC)
                              nc.gpsimd.affine_select(
                                  out=pt[:, b0:b1], in_=pt[:, b0:b1],
                                  compare_op=mybir.AluOpType.is_ge,
                                  fill=0.0, base=i0 + b0 - j * JC,
                                  pattern=[[1, b1 - b0]], channel_multiplier=-1)
                          pend.append((pt, j))
                      flush_pv(2)  # keep 1 pair in flight so PE never waits on ACT
                  flush_pv(0)

                  yt_sb = ytsb.tile([HS, TB], mybir.dt.bfloat16, tag="yts")
                  nc.vector.tensor_copy(yt_sb, yt_ps[0:HS, :])
                  nc.vector.tensor_copy(cs_sb[0:1, i0:i0 + TB], yt_ps[HS:HS + 1, :])
                  for q in range(4):
                      op_ps = opp.tile([128, TB], f32, tag="op")
                      nc.tensor.matmul(op_ps, yt_sb[:, q * 128:(q + 1) * 128], wp_sb)
                      ot = outp.tile([128, TB], mybir.dt.bfloat16, tag="ot")
                      nc.vector.tensor_copy(ot, op_ps)
                      nc.sync.dma_start(out=out_u.ap()[i0 + q * 128:i0 + (q + 1) * 128, :], in_=ot)

            nc.sync.dma_start(out=cs_out.ap(), in_=cs_sb)

    _legalize_waits(nc)
    return nc


_cached = {}


def _get_nc():
    if "nc" not in _cached:
        _cached["nc"] = _build_nc()
    return _cached["nc"]


def _prep_inputs(x, rope, W_attn, W_proj):
    bf16 = ml_dtypes.bfloat16
    xt = np.ascontiguousarray(x[0].T).astype(bf16)          # (C, T)
    cos = np.asarray(rope[..., 0], dtype=np.float32)        # (T, HS//2)
    sin = np.asarray(rope[..., 1], dtype=np.float32)
    cc64 = np.repeat(cos.T, 2, axis=0)                      # (HS, T)
    ss64 = np.repeat(sin.T, 2, axis=0)
    ss64[0::2, :] *= -1.0                                   # sign folded: even rows -sin
    cc = np.ascontiguousarray(np.concatenate([cc64, cc64], axis=0))   # (128, T)
    ss = np.ascontiguousarray(np.concatenate([ss64, ss64], axis=0))

    Wa = np.asarray(W_attn, dtype=np.float32)
    Wp = np.asarray(W_proj, dtype=np.float32)
    swap = np.arange(HS).reshape(-1, 2)[:, ::-1].reshape(-1)

    in_maps = []
    for h in range(NCORES):
        Wq = Wa[h * HS:(h + 1) * HS]                        # (HS, C)
        Wk = Wa[C + h * HS:C + (h + 1) * HS]
        Wv = Wa[2 * C + h * HS:2 * C + (h + 1) * HS]
        wqk = np.concatenate([Wq.T, Wk.T], axis=1).astype(bf16)        # (C, 128)
        wqks = np.concatenate([Wq[swap].T, Wk[swap].T], axis=1).astype(bf16)
        wv = np.ascontiguousarray(Wv.T).astype(bf16)                   # (C, HS)
        wp = np.ascontiguousarray(Wp[:, h * HS:(h + 1) * HS].T).astype(bf16)  # (HS, C)
        in_maps.append({
            "xt": xt, "wqk": wqk, "wqks": np.ascontiguousarray(wqks),
            "wv": wv, "wp": wp, "cc": cc, "ss": ss,
        })
    return in_maps


def run_cores(x, rope, W_attn, W_proj, trace=False):
    """Returns (list of per-core result dicts, BassKernelResults)."""
    nc = _get_nc()
    in_maps = _prep_inputs(x, rope, W_attn, W_proj)
    res = run_bass_kernel_spmd(nc, in_maps, list(range(NCORES)), trace=trace)
    return res


def kernel(x, rope, mask, W_attn, W_proj):
    res = run_cores(x, rope, W_attn, W_proj, trace=False)
    out = np.zeros((T, C), dtype=np.float32)
    for h in range(NCORES):
        r = res.results[h]
        cs = np.asarray(r["cs"], dtype=np.float32).reshape(T, 1)
        out += np.asarray(r["out_u"], dtype=np.float32) / cs
    return out.reshape(B, T, C).astype(np.float32)

